# revision 4
# baseline (speedup 1.0000x reference)
"""Partial-FC conv classifier kernel for 8 TRN2 NeuronCores.

Problem (hardcoded shapes): x [512, 512, 7, 7] f32, labels [512] i64,
weight [85742, 512, 1, 1] f32, bias [85742] f32.
reference: labels_unique = unique(labels, size=512, fill=0); w_sub =
weight[labels_unique]; logits = conv1x1(x, w_sub) + b_sub -> [512, 512, 7, 7].

Strategy: the unique-label gather is host-side data staging (it selects 512
rows / 1MB out of the 176MB table). The conv1x1 is a matmul
  out[u, (b,s)] = sum_c w_sub[u, c] * x[b, c, s].
Data-parallel over batch: core i computes batches [64*i, 64*(i+1)) with the
gathered weight replicated. Per core: [512x512] @ [512x3136] in fp16 with
fp32 PSUM accumulation (fp8 measures 4e-2 rel err -- fails the 2e-2 gate).

Trace-derived hardware model driving this schedule:
  - framework preamble ends ~6.3us; nothing starts before it.
  - a DMA packet covers one SBUF partition's contiguous run (<=4KB), each
    HWDGE queue streams ~one packet per ~10ns plus ~0.5us per-DMA gaps and
    16 tiny semaphore-write packets per DMA, and the first DMA pays ~1.5us
    of cold-start. Packet COUNT is everything: every unit here is a
    contiguous host array at 1-2 packets per partition, and the per-k
    "a" units embed w_k with the first x columns so one DMA (128 packets)
    gates the first matmul stage.
  - a [128,4]f32 bias DMA is 128 x 16B packets that would clog a HWDGE
    FIFO for ~1.5us, so bias rides the GpSimd software-DGE queue instead.
  - the PE HAM clock-gate needs ~3.5us of continuous matmul activity to
    reach full rate and falls back after ~1-2us idle: warm-up matmuls
    start immediately and hand over to a gap-free real stream.
  - cols 0:1792 run k-OUTER in 448-col sections (each stage gated on one
    arriving unit); cols 1792:3136 run k-inner/m-outer (x resident by
    then) so row blocks finish m-by-m and the post-matmul tail is one
    partition-split output piece.
"""

import numpy as np

import concourse.bass as bass  # noqa: F401  (registers types)
import concourse.mybir as mybir
import concourse.tile as tile
from concourse import bacc
from concourse.bass_utils import run_bass_kernel_spmd

N_CORES = 8
B = 512          # batch
C = 512          # channels (contraction)
HW = 49          # 7*7 spatial
U = 512          # unique labels (all distinct by construction)
B_LOC = B // N_CORES      # 64 batches per core
N_LOC = B_LOC * HW        # 3136 moving-dim columns per core
KT = C // 128             # 4 contraction tiles
MT = U // 128             # 4 output-partition tiles
SEC = 448                 # section width (one PSUM bank at fp32)
NSEC = N_LOC // SEC       # 7 sections
A_COLS = 896              # x cols in the per-k "a" units (sections 0-1)
B_COLS = 896              # x cols per k in the merged "b" unit (sections 2-3)
H_COLS = 1344             # x cols per k in the merged tail unit (sections 4-6)
KO_SECS = 4               # sections 0..3 run k-outer
N_WARM = 8                # warm-up matmuls bridging preamble -> first data

F32 = mybir.dt.float32
F16 = mybir.dt.float16

_MODULE = None


def _build_module():
    nc = bacc.Bacc("TRN2", target_bir_lowering=False, debug=False)
    # Host-packed, per-unit-contiguous layouts (see module docstring):
    #   aT[k] = [128, 512 w cols ++ 896 x cols]   (2816B/partition, 1 pkt)
    #   bT    = [128, 4k x 896 cols]              (7168B/partition)
    #   hT    = [128, 4k x 1344 cols]             (10752B/partition)
    aT = nc.dram_tensor("aT", [KT, 128, 512 + A_COLS], F16,
                        kind="ExternalInput").ap()
    bT = nc.dram_tensor("bT", [128, KT * B_COLS], F16,
                        kind="ExternalInput").ap()
    hT = nc.dram_tensor("hT", [128, KT * H_COLS], F16,
                        kind="ExternalInput").ap()
    bs = nc.dram_tensor("bs", [128, MT], F32, kind="ExternalInput").ap()
    out = nc.dram_tensor("out", [U, N_LOC], F16, kind="ExternalOutput").ap()

    with tile.TileContext(nc) as tc:
        with (
            tc.tile_pool(name="apool", bufs=KT) as apool,
            tc.tile_pool(name="xpool", bufs=2) as xpool,
            tc.tile_pool(name="bpool", bufs=1) as bpool,
            tc.tile_pool(name="scr", bufs=1) as scr,
            tc.tile_pool(name="opool", bufs=MT) as opool,
            tc.tile_pool(name="psum", bufs=8, space="PSUM") as psum,
        ):
            a_sb = [apool.tile([128, 512 + A_COLS], F16, tag="a",
                               name=f"a_{k}") for k in range(KT)]
            b_sb = xpool.tile([128, KT * B_COLS], F16, tag="b", name="b")
            h_sb = xpool.tile([128, KT * H_COLS], F16, tag="h", name="h")
            bias_sb = bpool.tile([128, MT], F32)

            # Input issue: both HWDGE rings in consumption order; the big
            # merged units are partition-split so both rings share them.
            nc.scalar.dma_start(a_sb[0][:], aT[0])
            nc.sync.dma_start(a_sb[1][:], aT[1])
            nc.scalar.dma_start(a_sb[2][:], aT[2])
            nc.sync.dma_start(a_sb[3][:], aT[3])
            nc.scalar.dma_start(b_sb[0:64, :], bT[0:64, :])
            nc.sync.dma_start(b_sb[64:128, :], bT[64:128, :])
            nc.scalar.dma_start(h_sb[0:64, :], hT[0:64, :])
            nc.sync.dma_start(h_sb[64:128, :], hT[64:128, :])
            # bias: 128 x 16B packets -> software DGE, off the hot rings
            nc.gpsimd.dma_start(bias_sb[:], bs[:])

            # Warm-ups: keep the PE busy (and the HAM clock-gate ramping)
            # from the moment the preamble ends until real data lands.
            scr_sb = scr.tile([128, 576], F16)
            nc.gpsimd.memset(scr_sb[:], 0.0)
            for i in range(N_WARM):
                ps_warm = psum.tile([128, SEC], F32, tag="ps", name=f"warm_{i}")
                nc.tensor.matmul(
                    ps_warm[:], scr_sb[:, :128], scr_sb[:, 128:576],
                    start=True, stop=True,
                )

            # Output staging: one full row-block per m-tile
            o_sb = [opool.tile([128, N_LOC], F16, tag="o", name=f"o_{m}")
                    for m in range(MT)]

            def w_slice(k, m):
                return a_sb[k][:, m * 128:(m + 1) * 128]

            def x_slice(k, c0, c1):
                if c1 <= A_COLS:
                    return a_sb[k][:, 512 + c0:512 + c1]
                if c1 <= A_COLS + B_COLS:
                    o = k * B_COLS - A_COLS
                    return b_sb[:, o + c0:o + c1]
                o = k * H_COLS - A_COLS - B_COLS
                return h_sb[:, o + c0:o + c1]

            def evict(ps, m, c0, c1, eng):
                dst = o_sb[m][:, c0:c1]
                if eng == "s":
                    nc.scalar.activation(
                        dst, ps[:], mybir.ActivationFunctionType.Identity,
                        bias=bias_sb[:, m:m + 1],
                    )
                else:
                    nc.vector.tensor_scalar_add(dst, ps[:], bias_sb[:, m:m + 1])

            # ---- sections 0..3 (cols 0:1792): k-outer, each stage gated
            # on one arriving unit; 4 PSUM banks per section.
            for s in range(KO_SECS):
                c0, c1 = s * SEC, (s + 1) * SEC
                ps_s = [psum.tile([128, SEC], F32, tag="ps",
                                  name=f"ps_{s}_{m}") for m in range(MT)]
                for k in range(KT):
                    xs = x_slice(k, c0, c1)
                    for m in range(MT):
                        nc.tensor.matmul(
                            ps_s[m][:], w_slice(k, m), xs,
                            start=(k == 0), stop=(k == KT - 1),
                        )
                for m in range(MT):
                    eng = "v" if (s == 0 or m % 2 == 0) else "s"
                    evict(ps_s[m], m, c0, c1, eng)
                if s == KO_SECS - 1:
                    # cols 0:1792 complete -> first piece (1 pkt/partition)
                    for m in range(MT):
                        dma_eng = nc.sync if m % 2 else nc.scalar
                        dma_eng.dma_start(out[m * 128:(m + 1) * 128, 0:1792],
                                          o_sb[m][:, 0:1792])

            # ---- sections 4..6 (cols 1792:3136): x resident; k-inner per
            # m so row blocks drain m-by-m with a short tail.
            for s in range(KO_SECS, NSEC):
                c0, c1 = s * SEC, (s + 1) * SEC
                for m in range(MT):
                    ps = psum.tile([128, SEC], F32, tag="ps",
                                   name=f"ps_{s}_{m}")
                    for k in range(KT):
                        nc.tensor.matmul(
                            ps[:], w_slice(k, m), x_slice(k, c0, c1),
                            start=(k == 0), stop=(k == KT - 1),
                        )
                    eng = "v" if m % 2 == 0 else "s"
                    evict(ps, m, c0, c1, eng)
                    if s == NSEC - 1:
                        # second piece per m right after its last evict;
                        # the very last row block is partition-split so
                        # both rings drain it in parallel.
                        r0, r1 = m * 128, (m + 1) * 128
                        if m < MT - 1:
                            dma_eng = nc.sync if m % 2 else nc.scalar
                            dma_eng.dma_start(out[r0:r1, 1792:3136],
                                              o_sb[m][:, 1792:3136])
                        else:
                            nc.scalar.dma_start(out[r0:r0 + 64, 1792:3136],
                                                o_sb[m][0:64, 1792:3136])
                            nc.sync.dma_start(out[r0 + 64:r1, 1792:3136],
                                              o_sb[m][64:128, 1792:3136])

    nc.compile()
    return nc


def _get_module():
    global _MODULE
    if _MODULE is None:
        _MODULE = _build_module()
    return _MODULE


def _prep_inputs(x, labels, weight, bias):
    x = np.asarray(x)
    labels = np.asarray(labels)
    weight = np.asarray(weight)
    bias = np.asarray(bias, dtype=np.float32)

    # jnp.unique(labels, size=B, fill_value=0): sorted unique, padded with 0.
    u = np.unique(labels)
    if u.size < U:
        u = np.concatenate([u, np.zeros(U - u.size, dtype=u.dtype)])
    u = u[:U]

    w_sub = weight.reshape(weight.shape[0], C)[u]                    # [U, C]
    # wk[k, p, m] = w_sub[m, 128k+p]
    wk = w_sub.T.astype(np.float16).reshape(KT, 128, U)
    b_sub = np.ascontiguousarray(bias[u].reshape(MT, 128).T)         # [128, MT]

    x16 = x.reshape(B, C, HW).astype(np.float16)
    in_maps = []
    for i in range(N_CORES):
        xi = x16[i * B_LOC:(i + 1) * B_LOC]
        # xt[k, p, col]: channel 128k+p, col = b*49+s
        xt = xi.transpose(1, 0, 2).reshape(KT, 128, N_LOC)
        aT = np.ascontiguousarray(
            np.concatenate([wk, xt[:, :, 0:A_COLS]], axis=2)
        )
        bT = np.ascontiguousarray(
            xt[:, :, A_COLS:A_COLS + B_COLS].transpose(1, 0, 2).reshape(
                128, KT * B_COLS)
        )
        hT = np.ascontiguousarray(
            xt[:, :, A_COLS + B_COLS:].transpose(1, 0, 2).reshape(
                128, KT * H_COLS)
        )
        in_maps.append({"aT": aT, "bT": bT, "hT": hT, "bs": b_sub})
    return in_maps


def _assemble_output(results):
    parts = []
    for i in range(N_CORES):
        oi = np.asarray(results[i]["out"]).astype(np.float32)  # [U, N_LOC]
        parts.append(
            np.ascontiguousarray(
                oi.reshape(U, B_LOC, HW).transpose(1, 0, 2)
            ).reshape(B_LOC, U, 7, 7)
        )
    return np.concatenate(parts, axis=0)


def run(x, labels, weight, bias, trace=False):
    in_maps = _prep_inputs(x, labels, weight, bias)
    nc = _get_module()
    res = run_bass_kernel_spmd(
        nc, in_maps, core_ids=list(range(N_CORES)), trace=trace
    )
    return _assemble_output(res.results), res


def kernel(x, labels, weight, bias):
    out, _ = run(x, labels, weight, bias, trace=False)
    return out
